# revision 14
# baseline (speedup 1.0000x reference)
"""Trainium2 Bass kernel for nn_DetectionLoss (2-class detection loss).

Computes, over B=2^24 rows of logits [B,2] and labels [B]:
  ce    = mean(-log_softmax(outputs)[label])
  pred  = argmax(outputs, axis=1)
  confusion counts TP/TN/FP/FN from (label, pred)
  CS    = M[pred, label] with M = [[0,1],[0,0]]  -> mean(CS) = FN/B
  loss  = ce + coeff(TP,TN,FP,FN) * mean(CS)

Device math (2 classes): with d = x1 - x0, h = label - 0.5 and
sgn = 1 - 2*label = -2*h:
  ce_row  = softplus(sgn*d) = log(1 + exp(-2*(d*h)))
  pred    = (d > 0)
  correct = (d*h > 0)            # prediction == label
Counts follow from three linear sums (n1 = sum(h) + B/2, p1 = sum(pred),
TP + TN = sum(correct)):
  TP = (sum(correct) + p1 + n1 - B) / 2, TN = sum(correct) - TP,
  FP = p1 - TP, FN = n1 - TP.

Engine split per chunk (all elementwise tensors bf16):
  DVE: h (2x), d (1x), u = d*h (2x), pred (4x), correct (4x)
  ACT: t = Exp(-2u), ce = Ln(1+t) with fused accum -> CE partial
  PE : sum(h) / sum(pred) / sum(correct) via ones-vector matmuls
       accumulated in PSUM across all chunks (DVE reductions run at 1x,
       the tensor engine is otherwise idle)
The tiny per-core partials ([nchunks,128] CE + [3*512] counts) are
combined on the host; count arithmetic is exact (half-integers in fp32).

Sharding: data-parallel over the batch dim across 8 NeuronCores.
"""

import numpy as np

import concourse.bass as bass
import concourse.mybir as mybir
import concourse.tile as tile
from concourse.bass_utils import run_bass_kernel_spmd

N_CORES = 8
P = 128
LAMBD = 0.5
NCHUNKS = 8
MMN = 512  # matmul rhs free-dim tile (one PSUM bank)

_cache = {}

_MAX_WAITS = 1  # this walrus build rejects >1 embedded sync-wait per instruction


def _split_multiwaits(nc):
    """Walrus in this container can't encode instructions with multiple
    sync waits; hoist all but the last into standalone EventSemaphore
    waits on the same engine immediately before the instruction."""
    n = [0]

    def fix_block(blk):
        new_insts = []
        for ins in blk.instructions:
            si = ins.sync_info
            if si is not None and si.on_wait and len(si.on_wait) > _MAX_WAITS:
                waits = list(si.on_wait)
                for w in waits[: -_MAX_WAITS]:
                    n[0] += 1
                    ev = mybir.InstEventSemaphore(
                        name=f"I-waitsplit-{n[0]}",
                        ins=[],
                        outs=[],
                        sync_info=mybir.SyncInfo(on_wait=[w], on_update=[]),
                    )
                    ev.engine = ins.engine
                    new_insts.append(ev)
                si.on_wait = waits[-_MAX_WAITS:]
            new_insts.append(ins)
        blk.instructions = new_insts

    for fn in nc.m.functions:
        for blk in fn.blocks:
            fix_block(blk)


def _build(rows_per_core: int, nchunks: int, lab64: bool):
    """Build the per-core Bass module. All cores run the same program on
    their own shard (pure data parallel, no collectives)."""
    key = (rows_per_core, nchunks, lab64)
    if key in _cache:
        return _cache[key]

    F = rows_per_core // (P * nchunks)  # rows per partition per chunk
    assert rows_per_core == F * P * nchunks, (rows_per_core, nchunks)
    assert F % MMN == 0

    nc = bass.Bass(trn_type="TRN2")
    dtf = mybir.dt.float32
    dti = mybir.dt.int32
    dtb = mybir.dt.bfloat16
    Op = mybir.AluOpType
    Act = mybir.ActivationFunctionType

    LW = 2 if lab64 else 1  # int32 words per label
    x = nc.dram_tensor("x", [nchunks, P, 2 * F], dtf, kind="ExternalInput")
    lab = nc.dram_tensor("lab", [nchunks, P, LW * F], dti, kind="ExternalInput")
    acc_ce = nc.dram_tensor("acc_ce", [nchunks, P, 1], dtf, kind="ExternalOutput")
    acc_cnt = nc.dram_tensor("acc_cnt", [1, 3 * MMN], dtf, kind="ExternalOutput")

    with tile.TileContext(nc) as tc:
        with (
            tc.tile_pool(name="io", bufs=3) as io_pool,
            tc.tile_pool(name="mid", bufs=3) as mid,
            tc.tile_pool(name="junk", bufs=2) as junk,
            tc.tile_pool(name="st", bufs=3) as stp,
            tc.tile_pool(name="singles", bufs=1) as singles,
            tc.tile_pool(name="ps", bufs=1, space="PSUM") as psp,
        ):
            ones = singles.tile([P, 1], dtb)
            nc.vector.memset(ones, 1.0)
            ps_h = psp.tile([1, MMN], dtf, tag="ps_h")
            ps_p = psp.tile([1, MMN], dtf, tag="ps_p")
            ps_e = psp.tile([1, MMN], dtf, tag="ps_e")

            nslab = F // MMN
            for c in range(nchunks):
                xt = io_pool.tile([P, 2 * F], dtf, tag="xt")
                nc.sync.dma_start(out=xt, in_=x[c])
                lt = io_pool.tile([P, LW * F], dti, tag="lt")
                nc.sync.dma_start(out=lt, in_=lab[c])
                st = stp.tile([P, 1], dtf, tag="st")

                xp = xt.rearrange("p (f two) -> p f two", two=2)
                if lab64:
                    # int64 labels as little-endian int32 pairs; low word
                    # (stride 2) holds the value.
                    lv = lt.rearrange("p (f two) -> p f two", two=2)[:, :, 0]
                else:
                    lv = lt[:]

                # h = label - 0.5 in {-0.5,+0.5}
                h = mid.tile([P, F], dtb, tag="h")
                nc.vector.tensor_scalar(
                    out=h, in0=lv, scalar1=0.5, scalar2=None, op0=Op.subtract
                )
                # d = x1 - x0
                d = mid.tile([P, F], dtb, tag="d")
                nc.vector.tensor_sub(out=d, in0=xp[:, :, 1], in1=xp[:, :, 0])
                # u = d*h  (sign-folded logit margin; ce_row = log1p(exp(-2u)))
                u = mid.tile([P, F], dtb, tag="u")
                nc.vector.tensor_mul(out=u, in0=d, in1=h)
                # pred = (d > 0), correct = (u > 0)
                pred = mid.tile([P, F], dtb, tag="pred")
                nc.vector.tensor_scalar(
                    out=pred, in0=d, scalar1=0.0, scalar2=None, op0=Op.is_gt
                )
                e = mid.tile([P, F], dtb, tag="e")
                nc.vector.tensor_scalar(
                    out=e, in0=u, scalar1=0.0, scalar2=None, op0=Op.is_gt
                )

                # CE partial on ACT: t = exp(-2u); ce = ln(1+t), accum sum
                t = mid.tile([P, F], dtb, tag="t")
                nc.scalar.activation(out=t, in_=u, func=Act.Exp, scale=-2.0)
                j3 = junk.tile([P, F], dtf, tag="j3")
                nc.scalar.activation(
                    out=j3,
                    in_=t,
                    func=Act.Ln,
                    bias=1.0,
                    scale=1.0,
                    accum_out=st[:, 0:1],
                )

                # Count partials on PE: ones^T @ slab accumulates per-column
                # sums into PSUM across all chunks.
                for k in range(nslab):
                    sl = slice(k * MMN, (k + 1) * MMN)
                    first = c == 0 and k == 0
                    last = c == nchunks - 1 and k == nslab - 1
                    nc.tensor.matmul(
                        ps_h[:, :], ones, h[:, sl], start=first, stop=last
                    )
                    nc.tensor.matmul(
                        ps_p[:, :], ones, pred[:, sl], start=first, stop=last
                    )
                    nc.tensor.matmul(
                        ps_e[:, :], ones, e[:, sl], start=first, stop=last
                    )

                nc.sync.dma_start(out=acc_ce[c], in_=st)

            cnt_sb = singles.tile([1, 3 * MMN], dtf)
            nc.vector.tensor_copy(out=cnt_sb[:, 0 * MMN : 1 * MMN], in_=ps_h)
            nc.vector.tensor_copy(out=cnt_sb[:, 1 * MMN : 2 * MMN], in_=ps_p)
            nc.vector.tensor_copy(out=cnt_sb[:, 2 * MMN : 3 * MMN], in_=ps_e)
            nc.sync.dma_start(out=acc_cnt[:], in_=cnt_sb)

    _cache[key] = (nc, F)
    return nc, F


def _combine(acc_ce: np.ndarray, acc_cnt: np.ndarray, B: int) -> np.ndarray:
    """Host-side scalar epilogue.

    acc_ce: [n_cores, nchunks, P, 1] f32 CE partial sums.
    acc_cnt: [n_cores, 1, 3*MMN] f32 PE-reduced count partials
             (columns: sum(h) | sum(pred) | sum(correct)).
    Counts are exact half-integers in fp32 at every stage."""
    CE = acc_ce.astype(np.float64).sum()
    cnt = acc_cnt.astype(np.float64).reshape(-1, 3, MMN).sum(axis=(0, 2))
    H1, p1, C = cnt
    n1 = H1 + B / 2.0  # labels == 1
    TP = (C + p1 + n1 - B) / 2.0
    TN = C - TP
    FP = p1 - TP
    FN = n1 - TP

    ce = CE / B
    mean_cs = FN / B
    nonzero = (TP > 0) and (TN > 0) and (FP > 0) and (FN > 0)
    ratio = (TP / max(TP + FN, 1.0)) * (FP / max(FP + TN, 1.0))
    if nonzero:
        coeff = -LAMBD * np.log(np.sqrt(max(ratio, 1e-30)))
    else:
        coeff = LAMBD
    return np.array(ce + coeff * mean_cs, dtype=np.float32)


def run(outputs: np.ndarray, labels: np.ndarray):
    """Run on 8 cores; returns (loss, BassKernelResults)."""
    outputs = np.asarray(outputs)
    labels = np.asarray(labels)
    B = outputs.shape[0]
    assert outputs.shape == (B, 2) and labels.shape == (B,)
    assert B % N_CORES == 0
    S = B // N_CORES

    lab64 = labels.dtype.itemsize == 8
    nc, F = _build(S, NCHUNKS, lab64)
    _split_multiwaits(nc)  # idempotent; CoreSim needs the unsplit module
    LW = 2 if lab64 else 1

    in_maps = []
    for i in range(N_CORES):
        xs = np.ascontiguousarray(outputs[i * S : (i + 1) * S], dtype=np.float32)
        xs = xs.reshape(NCHUNKS, P, 2 * F)
        ls = np.ascontiguousarray(labels[i * S : (i + 1) * S])
        ls = ls.view(np.int32).reshape(NCHUNKS, P, LW * F)
        in_maps.append({"x": xs, "lab": ls})

    res = run_bass_kernel_spmd(nc, in_maps, core_ids=list(range(N_CORES)))
    acc_ce = np.stack([r["acc_ce"] for r in res.results])
    acc_cnt = np.stack([r["acc_cnt"] for r in res.results])
    return _combine(acc_ce, acc_cnt, B), res


def kernel(outputs: np.ndarray, labels: np.ndarray) -> np.ndarray:
    return run(outputs, labels)[0]


# revision 16
# speedup vs baseline: 1.4374x; 1.4374x over previous
"""Trainium2 Bass kernel for nn_DetectionLoss (2-class detection loss).

Computes, over B=2^24 rows of logits [B,2] and labels [B]:
  ce    = mean(-log_softmax(outputs)[label])
  pred  = argmax(outputs, axis=1)
  confusion counts TP/TN/FP/FN from (label, pred)
  CS    = M[pred, label] with M = [[0,1],[0,0]]  -> mean(CS) = FN/B
  loss  = ce + coeff(TP,TN,FP,FN) * mean(CS)

Device math (2 classes): with d = x1 - x0, h = label - 0.5 and
sgn = 1 - 2*label = -2*h:
  ce_row  = softplus(sgn*d) = log(1 + exp(-2*(d*h)))
  pred    = (d > 0)
  correct = (d*h > 0)            # prediction == label
Counts follow from three linear sums (n1 = sum(h) + B/2, p1 = sum(pred),
TP + TN = sum(correct)):
  TP = (sum(correct) + p1 + n1 - B) / 2, TN = sum(correct) - TP,
  FP = p1 - TP, FN = n1 - TP.

Engine split per chunk (all elementwise tensors bf16):
  DVE: h (2x), d (1x), u = d*h (2x), pred (4x), correct (4x)
  ACT: t = Exp(-2u), ce = Ln(1+t) with fused accum -> CE partial
  PE : sum(h) / sum(pred) / sum(correct) via ones-vector matmuls
       accumulated in PSUM across all chunks (DVE reductions run at 1x,
       the tensor engine is otherwise idle)
The tiny per-core partials ([nchunks,128] CE + [3*512] counts) are
combined on the host; count arithmetic is exact (half-integers in fp32).

Sharding: data-parallel over the batch dim across 8 NeuronCores.
"""

import numpy as np

import concourse.bass as bass
import concourse.mybir as mybir
import concourse.tile as tile
from concourse.bass_utils import run_bass_kernel_spmd

N_CORES = 8
P = 128
LAMBD = 0.5
NCHUNKS = 8
MMN = 512  # matmul rhs free-dim tile (one PSUM bank)

_cache = {}

_MAX_WAITS = 1  # this walrus build rejects >1 embedded sync-wait per instruction


def _split_multiwaits(nc):
    """Walrus in this container can't encode instructions with multiple
    sync waits; hoist all but the last into standalone EventSemaphore
    waits on the same engine immediately before the instruction."""
    n = [0]

    def fix_block(blk):
        new_insts = []
        for ins in blk.instructions:
            si = ins.sync_info
            if si is not None and si.on_wait and len(si.on_wait) > _MAX_WAITS:
                waits = list(si.on_wait)
                for w in waits[: -_MAX_WAITS]:
                    n[0] += 1
                    ev = mybir.InstEventSemaphore(
                        name=f"I-waitsplit-{n[0]}",
                        ins=[],
                        outs=[],
                        sync_info=mybir.SyncInfo(on_wait=[w], on_update=[]),
                    )
                    ev.engine = ins.engine
                    new_insts.append(ev)
                si.on_wait = waits[-_MAX_WAITS:]
            new_insts.append(ins)
        blk.instructions = new_insts

    for fn in nc.m.functions:
        for blk in fn.blocks:
            fix_block(blk)


def _build(rows_per_core: int, nchunks: int, lab64: bool):
    """Build the per-core Bass module. All cores run the same program on
    their own shard (pure data parallel, no collectives)."""
    key = (rows_per_core, nchunks, lab64)
    if key in _cache:
        return _cache[key]

    F = rows_per_core // (P * nchunks)  # rows per partition per chunk
    assert rows_per_core == F * P * nchunks, (rows_per_core, nchunks)
    assert F % MMN == 0

    nc = bass.Bass(trn_type="TRN2")
    dtf = mybir.dt.float32
    dti = mybir.dt.int32
    dtb = mybir.dt.bfloat16
    Op = mybir.AluOpType
    Act = mybir.ActivationFunctionType

    LW = 2 if lab64 else 1  # int32 words per label
    x = nc.dram_tensor("x", [nchunks, P, 2 * F], dtf, kind="ExternalInput")
    lab = nc.dram_tensor("lab", [nchunks, P, LW * F], dti, kind="ExternalInput")
    acc_ce = nc.dram_tensor("acc_ce", [nchunks, P, 2], dtf, kind="ExternalOutput")
    acc_cnt = nc.dram_tensor("acc_cnt", [1, 2 * MMN], dtf, kind="ExternalOutput")

    with tile.TileContext(nc) as tc:
        with (
            tc.tile_pool(name="io", bufs=4) as io_pool,
            tc.tile_pool(name="mid", bufs=3) as mid,
            tc.tile_pool(name="junk", bufs=2) as junk,
            tc.tile_pool(name="st", bufs=3) as stp,
            tc.tile_pool(name="singles", bufs=1) as singles,
            tc.tile_pool(name="ps", bufs=1, space="PSUM") as psp,
        ):
            ones = singles.tile([P, 1], dtb)
            nc.vector.memset(ones, 1.0)
            ps_s = psp.tile([1, MMN], dtf, tag="ps_s")
            ps_e = psp.tile([1, MMN], dtf, tag="ps_e")

            nslab = F // MMN
            for c in range(nchunks):
                # Loads ride the SP HWDGE ring exclusively; stores go out on
                # the ACT ring so a chunk's accum store can't head-of-line
                # block later prefetches.
                xt = io_pool.tile([P, 2 * F], dtf, tag="xt")
                nc.sync.dma_start(out=xt, in_=x[c])
                lt = io_pool.tile([P, LW * F], dti, tag="lt")
                nc.sync.dma_start(out=lt, in_=lab[c])
                st = stp.tile([P, 2], dtf, tag="st")

                xp = xt.rearrange("p (f two) -> p f two", two=2)
                if lab64:
                    # int64 labels as little-endian int32 pairs; low word
                    # (stride 2) holds the value.
                    lv = lt.rearrange("p (f two) -> p f two", two=2)[:, :, 0]
                else:
                    lv = lt[:]

                # h = label - 0.5 in {-0.5,+0.5}
                h = mid.tile([P, F], dtb, tag="h")
                nc.vector.tensor_scalar(
                    out=h, in0=lv, scalar1=0.5, scalar2=None, op0=Op.subtract
                )
                # d = x1 - x0
                d = mid.tile([P, F], dtb, tag="d")
                nc.vector.tensor_sub(out=d, in0=xp[:, :, 1], in1=xp[:, :, 0])
                # u = d*h  (sign-folded logit margin; ce_row = log1p(exp(-2u)))
                u = mid.tile([P, F], dtb, tag="u")
                nc.vector.tensor_mul(out=u, in0=d, in1=h)
                # s = pred + h with pred = (d > 0); correct = (u > 0)
                s = mid.tile([P, F], dtb, tag="s")
                nc.vector.scalar_tensor_tensor(
                    out=s, in0=d, scalar=0.0, in1=h, op0=Op.is_gt, op1=Op.add
                )
                e = mid.tile([P, F], dtb, tag="e")
                nc.vector.tensor_scalar(
                    out=e, in0=u, scalar1=0.0, scalar2=None, op0=Op.is_gt
                )

                # ACT: H1 partial (sum of h per partition), then the CE chain
                # t = exp(-2u); ce = ln(1+t) with fused accum.
                jh = junk.tile([P, F], dtb, tag="jh")
                nc.scalar.activation(
                    out=jh, in_=h, func=Act.Identity, accum_out=st[:, 1:2]
                )
                t = mid.tile([P, F], dtb, tag="t")
                nc.scalar.activation(out=t, in_=u, func=Act.Exp, scale=-2.0)
                j3 = junk.tile([P, F], dtf, tag="j3")
                nc.scalar.activation(
                    out=j3,
                    in_=t,
                    func=Act.Ln,
                    bias=1.0,
                    scale=1.0,
                    accum_out=st[:, 0:1],
                )

                # Count partials on PE: ones^T @ slab accumulates per-column
                # sums into PSUM across all chunks.
                for k in range(nslab):
                    sl = slice(k * MMN, (k + 1) * MMN)
                    first = c == 0 and k == 0
                    last = c == nchunks - 1 and k == nslab - 1
                    nc.tensor.matmul(
                        ps_s[:, :], ones, s[:, sl], start=first, stop=last
                    )
                    nc.tensor.matmul(
                        ps_e[:, :], ones, e[:, sl], start=first, stop=last
                    )

                nc.scalar.dma_start(out=acc_ce[c], in_=st)

            cnt_sb = singles.tile([1, 2 * MMN], dtf)
            nc.vector.tensor_copy(out=cnt_sb[:, 0 * MMN : 1 * MMN], in_=ps_s)
            nc.vector.tensor_copy(out=cnt_sb[:, 1 * MMN : 2 * MMN], in_=ps_e)
            nc.scalar.dma_start(out=acc_cnt[:], in_=cnt_sb)

    _cache[key] = (nc, F)
    return nc, F


def _combine(acc_ce: np.ndarray, acc_cnt: np.ndarray, B: int) -> np.ndarray:
    """Host-side scalar epilogue.

    acc_ce: [n_cores, nchunks, P, 2] f32 partials (col0: CE, col1: sum(h)).
    acc_cnt: [n_cores, 1, 2*MMN] f32 PE-reduced count partials
             (columns: sum(pred + h) | sum(correct)).
    Counts are exact half-integers in fp32 at every stage."""
    CE = acc_ce[..., 0].astype(np.float64).sum()
    H1 = acc_ce[..., 1].astype(np.float64).sum()
    cnt = acc_cnt.astype(np.float64).reshape(-1, 2, MMN).sum(axis=(0, 2))
    S1, C = cnt
    n1 = H1 + B / 2.0  # labels == 1
    p1 = S1 - H1  # preds == 1
    TP = (C + p1 + n1 - B) / 2.0
    TN = C - TP
    FP = p1 - TP
    FN = n1 - TP

    ce = CE / B
    mean_cs = FN / B
    nonzero = (TP > 0) and (TN > 0) and (FP > 0) and (FN > 0)
    ratio = (TP / max(TP + FN, 1.0)) * (FP / max(FP + TN, 1.0))
    if nonzero:
        coeff = -LAMBD * np.log(np.sqrt(max(ratio, 1e-30)))
    else:
        coeff = LAMBD
    return np.array(ce + coeff * mean_cs, dtype=np.float32)


def run(outputs: np.ndarray, labels: np.ndarray):
    """Run on 8 cores; returns (loss, BassKernelResults)."""
    outputs = np.asarray(outputs)
    labels = np.asarray(labels)
    B = outputs.shape[0]
    assert outputs.shape == (B, 2) and labels.shape == (B,)
    assert B % N_CORES == 0
    S = B // N_CORES

    lab64 = labels.dtype.itemsize == 8
    nc, F = _build(S, NCHUNKS, lab64)
    _split_multiwaits(nc)  # idempotent; CoreSim needs the unsplit module
    LW = 2 if lab64 else 1

    in_maps = []
    for i in range(N_CORES):
        xs = np.ascontiguousarray(outputs[i * S : (i + 1) * S], dtype=np.float32)
        xs = xs.reshape(NCHUNKS, P, 2 * F)
        ls = np.ascontiguousarray(labels[i * S : (i + 1) * S])
        ls = ls.view(np.int32).reshape(NCHUNKS, P, LW * F)
        in_maps.append({"x": xs, "lab": ls})

    res = run_bass_kernel_spmd(nc, in_maps, core_ids=list(range(N_CORES)))
    acc_ce = np.stack([r["acc_ce"] for r in res.results])
    acc_cnt = np.stack([r["acc_cnt"] for r in res.results])
    return _combine(acc_ce, acc_cnt, B), res


def kernel(outputs: np.ndarray, labels: np.ndarray) -> np.ndarray:
    return run(outputs, labels)[0]


# revision 18
# speedup vs baseline: 1.6541x; 1.1508x over previous
"""Trainium2 Bass kernel for nn_DetectionLoss (2-class detection loss).

Computes, over B=2^24 rows of logits [B,2] and labels [B]:
  ce    = mean(-log_softmax(outputs)[label])
  pred  = argmax(outputs, axis=1)
  confusion counts TP/TN/FP/FN from (label, pred)
  CS    = M[pred, label] with M = [[0,1],[0,0]]  -> mean(CS) = FN/B
  loss  = ce + coeff(TP,TN,FP,FN) * mean(CS)

Device math (2 classes): with d = x1 - x0, h = label - 0.5 and
sgn = 1 - 2*label = -2*h:
  ce_row  = softplus(sgn*d) = log(1 + exp(-2*(d*h)))
  pred    = (d > 0)
  correct = (d*h > 0)            # prediction == label
Counts follow from three linear sums (n1 = sum(h) + B/2, p1 = sum(pred),
TP + TN = sum(correct)):
  TP = (sum(correct) + p1 + n1 - B) / 2, TN = sum(correct) - TP,
  FP = p1 - TP, FN = n1 - TP.

Engine split per chunk (elementwise tensors bf16):
  DVE: h (2x), d (1x), u = d*h (2x), pred (4x), correct (4x)
  ACT: t = Exp(-2u); ce = Ln(1+t) with fused accum -> per-chunk CE partial
  PE : sum(h) / sum(pred) / sum(correct) via ones-vector matmuls
       accumulated in PSUM across all chunks (DVE reductions run at 1x;
       the tensor engine is otherwise idle)
Inputs stream through SBUF in variable-size chunks (small at both ends to
shorten pipeline fill/drain latency; ~3 MiB in the middle to keep DMA at
line rate). The tiny per-core partials are combined on the host; count
arithmetic is exact (half-integers in fp32).

Sharding: data-parallel over the batch dim across 8 NeuronCores.
"""

import numpy as np

import concourse.bass as bass
import concourse.mybir as mybir
import concourse.tile as tile
from concourse.bass_utils import run_bass_kernel_spmd

N_CORES = 8
P = 128
LAMBD = 0.5
MMN = 512  # matmul rhs free-dim tile (one PSUM bank)

_cache = {}

_MAX_WAITS = 1  # this walrus build rejects >1 embedded sync-wait per instruction


def _split_multiwaits(nc):
    """Walrus in this container can't encode instructions with multiple
    sync waits; hoist all but the last into standalone EventSemaphore
    waits on the same engine immediately before the instruction."""
    n = [0]

    def fix_block(blk):
        new_insts = []
        for ins in blk.instructions:
            si = ins.sync_info
            if si is not None and si.on_wait and len(si.on_wait) > _MAX_WAITS:
                waits = list(si.on_wait)
                for w in waits[: -_MAX_WAITS]:
                    n[0] += 1
                    ev = mybir.InstEventSemaphore(
                        name=f"I-waitsplit-{n[0]}",
                        ins=[],
                        outs=[],
                        sync_info=mybir.SyncInfo(on_wait=[w], on_update=[]),
                    )
                    ev.engine = ins.engine
                    new_insts.append(ev)
                si.on_wait = waits[-_MAX_WAITS:]
            new_insts.append(ins)
        blk.instructions = new_insts

    for fn in nc.m.functions:
        for blk in fn.blocks:
            fix_block(blk)


def _chunk_plan(rpp: int):
    """Rows-per-partition per chunk. Small chunks at both ends shorten the
    pipeline fill (first compute can't start before chunk 0 lands) and the
    tail (last chunk's compute latency after the final DMA byte)."""
    if rpp == 16384:
        plan = [512, 1024, 1536] + [2048] * 5 + [1536, 1024, 512]
    else:
        # small test sizes: four equal chunks
        assert rpp % 4 == 0
        plan = [rpp // 4] * 4
    assert sum(plan) == rpp and all(f % 256 == 0 for f in plan)
    return plan


def _build(rows_per_core: int, lab64: bool):
    """Build the per-core Bass module. All cores run the same program on
    their own shard (pure data parallel, no collectives)."""
    key = (rows_per_core, lab64)
    if key in _cache:
        return _cache[key]

    assert rows_per_core % P == 0
    rpp = rows_per_core // P  # rows per partition
    plan = _chunk_plan(rpp)
    nch = len(plan)
    fmax = max(plan)

    nc = bass.Bass(trn_type="TRN2")
    dtf = mybir.dt.float32
    dti = mybir.dt.int32
    dtb = mybir.dt.bfloat16
    Op = mybir.AluOpType
    Act = mybir.ActivationFunctionType

    LW = 2 if lab64 else 1  # int32 words per label
    x = nc.dram_tensor("x", [P, 2 * rpp], dtf, kind="ExternalInput")
    lab = nc.dram_tensor("lab", [P, LW * rpp], dti, kind="ExternalInput")
    acc_ce = nc.dram_tensor("acc_ce", [P, nch], dtf, kind="ExternalOutput")
    acc_cnt = nc.dram_tensor("acc_cnt", [1, 3 * MMN], dtf, kind="ExternalOutput")

    with tile.TileContext(nc) as tc:
        with (
            tc.tile_pool(name="io", bufs=4) as io_pool,
            tc.tile_pool(name="mid", bufs=3) as mid,
            tc.tile_pool(name="junk", bufs=2) as junk,
            tc.tile_pool(name="singles", bufs=1) as singles,
            tc.tile_pool(name="ps", bufs=1, space="PSUM") as psp,
        ):
            ones = singles.tile([P, 1], dtb)
            nc.vector.memset(ones, 1.0)
            st = singles.tile([P, nch], dtf)
            ps_h = psp.tile([1, MMN], dtf, tag="ps_h")
            ps_p = psp.tile([1, MMN], dtf, tag="ps_p")
            ps_e = psp.tile([1, MMN], dtf, tag="ps_e")

            r0 = 0
            for c, F in enumerate(plan):
                r1 = r0 + F
                xt_full = io_pool.tile([P, 2 * fmax], dtf, tag="xt")
                xt = xt_full[:, : 2 * F]
                nc.sync.dma_start(out=xt, in_=x[:, 2 * r0 : 2 * r1])
                lt_full = io_pool.tile([P, LW * fmax], dti, tag="lt")
                lt = lt_full[:, : LW * F]
                nc.sync.dma_start(out=lt, in_=lab[:, LW * r0 : LW * r1])

                xp = xt.rearrange("p (f two) -> p f two", two=2)
                if lab64:
                    # int64 labels as little-endian int32 pairs; low word
                    # (stride 2) holds the value.
                    lv = lt.rearrange("p (f two) -> p f two", two=2)[:, :, 0]
                else:
                    lv = lt

                # h = label - 0.5 in {-0.5,+0.5}
                h_full = mid.tile([P, fmax], dtb, tag="h")
                h = h_full[:, :F]
                nc.vector.tensor_scalar(
                    out=h, in0=lv, scalar1=0.5, scalar2=None, op0=Op.subtract
                )
                # d = x1 - x0
                d_full = mid.tile([P, fmax], dtb, tag="d")
                d = d_full[:, :F]
                nc.vector.tensor_sub(out=d, in0=xp[:, :, 1], in1=xp[:, :, 0])
                # u = d*h  (sign-folded logit margin; ce_row = log1p(exp(-2u)))
                u_full = mid.tile([P, fmax], dtb, tag="u")
                u = u_full[:, :F]
                nc.vector.tensor_mul(out=u, in0=d, in1=h)
                # pred = (d > 0), correct = (u > 0)
                pred_full = mid.tile([P, fmax], dtb, tag="pred")
                pred = pred_full[:, :F]
                nc.vector.tensor_scalar(
                    out=pred, in0=d, scalar1=0.0, scalar2=None, op0=Op.is_gt
                )
                e_full = mid.tile([P, fmax], dtb, tag="e")
                e = e_full[:, :F]
                nc.vector.tensor_scalar(
                    out=e, in0=u, scalar1=0.0, scalar2=None, op0=Op.is_gt
                )

                # CE partial on ACT: t = exp(-2u); ce = ln(1+t), accum sum
                # into this chunk's column of the persistent st tile.
                t_full = mid.tile([P, fmax], dtb, tag="t")
                t = t_full[:, :F]
                nc.scalar.activation(out=t, in_=u, func=Act.Exp, scale=-2.0)
                j3_full = junk.tile([P, fmax], dtf, tag="j3")
                j3 = j3_full[:, :F]
                nc.scalar.activation(
                    out=j3,
                    in_=t,
                    func=Act.Ln,
                    bias=1.0,
                    scale=1.0,
                    accum_out=st[:, c : c + 1],
                )

                # Count partials on PE: ones^T @ slab accumulates per-column
                # sums into PSUM across all chunks.
                nslab = (F + MMN - 1) // MMN
                for k in range(nslab):
                    sl = slice(k * MMN, min((k + 1) * MMN, F))
                    w = sl.stop - sl.start
                    first = c == 0 and k == 0
                    last = c == nch - 1 and k == nslab - 1
                    nc.tensor.matmul(
                        ps_h[:, :w], ones, h[:, sl], start=first, stop=last
                    )
                    nc.tensor.matmul(
                        ps_p[:, :w], ones, pred[:, sl], start=first, stop=last
                    )
                    nc.tensor.matmul(
                        ps_e[:, :w], ones, e[:, sl], start=first, stop=last
                    )
                r0 = r1

            nc.scalar.dma_start(out=acc_ce[:], in_=st)
            cnt_sb = singles.tile([1, 3 * MMN], dtf)
            nc.vector.tensor_copy(out=cnt_sb[:, 0 * MMN : 1 * MMN], in_=ps_h)
            nc.vector.tensor_copy(out=cnt_sb[:, 1 * MMN : 2 * MMN], in_=ps_p)
            nc.vector.tensor_copy(out=cnt_sb[:, 2 * MMN : 3 * MMN], in_=ps_e)
            nc.scalar.dma_start(out=acc_cnt[:], in_=cnt_sb)

    _cache[key] = (nc, nch)
    return nc, nch


def _combine(acc_ce: np.ndarray, acc_cnt: np.ndarray, B: int) -> np.ndarray:
    """Host-side scalar epilogue.

    acc_ce: [n_cores, P, nch] f32 CE partial sums.
    acc_cnt: [n_cores, 1, 3*MMN] f32 PE-reduced count partials
             (columns: sum(h) | sum(pred) | sum(correct)).
    Counts are exact half-integers in fp32 at every stage."""
    CE = acc_ce.astype(np.float64).sum()
    cnt = acc_cnt.astype(np.float64).reshape(-1, 3, MMN).sum(axis=(0, 2))
    H1, p1, C = cnt
    n1 = H1 + B / 2.0  # labels == 1
    TP = (C + p1 + n1 - B) / 2.0
    TN = C - TP
    FP = p1 - TP
    FN = n1 - TP

    ce = CE / B
    mean_cs = FN / B
    nonzero = (TP > 0) and (TN > 0) and (FP > 0) and (FN > 0)
    ratio = (TP / max(TP + FN, 1.0)) * (FP / max(FP + TN, 1.0))
    if nonzero:
        coeff = -LAMBD * np.log(np.sqrt(max(ratio, 1e-30)))
    else:
        coeff = LAMBD
    return np.array(ce + coeff * mean_cs, dtype=np.float32)


def run(outputs: np.ndarray, labels: np.ndarray):
    """Run on 8 cores; returns (loss, BassKernelResults)."""
    outputs = np.asarray(outputs)
    labels = np.asarray(labels)
    B = outputs.shape[0]
    assert outputs.shape == (B, 2) and labels.shape == (B,)
    assert B % (N_CORES * P) == 0
    S = B // N_CORES
    rpp = S // P

    lab64 = labels.dtype.itemsize == 8
    nc, nch = _build(S, lab64)
    _split_multiwaits(nc)  # idempotent; CoreSim needs the unsplit module
    LW = 2 if lab64 else 1

    in_maps = []
    for i in range(N_CORES):
        xs = np.ascontiguousarray(outputs[i * S : (i + 1) * S], dtype=np.float32)
        xs = xs.reshape(P, 2 * rpp)
        ls = np.ascontiguousarray(labels[i * S : (i + 1) * S])
        ls = ls.view(np.int32).reshape(P, LW * rpp)
        in_maps.append({"x": xs, "lab": ls})

    res = run_bass_kernel_spmd(nc, in_maps, core_ids=list(range(N_CORES)))
    acc_ce = np.stack([r["acc_ce"] for r in res.results])
    acc_cnt = np.stack([r["acc_cnt"] for r in res.results])
    return _combine(acc_ce, acc_cnt, B), res


def kernel(outputs: np.ndarray, labels: np.ndarray) -> np.ndarray:
    return run(outputs, labels)[0]
